# revision 3
# baseline (speedup 1.0000x reference)
"""Multi-head cross-attention (MHAForCrossFusion) on 8 Trainium2 cores.

v5: hybrid batch x head tensor-parallel (cores 0-3 batch 0, cores 4-7
batch 1; 4 heads / 256 features per core), all matmuls bf16, host sums
the 4 partial out-projections per batch (+ bo).

q/k/v are transposed to feature-major [D, L] on the HOST (like the
weight layouts), so the device sees plain fast DMAs instead of XBAR
DMA-transposes -- the q/k/v feature-major tiles live resident in SBUF.

Device program per core:
 - qm/km feature-major [256f, 2048t] = Wx_slice.T @ xT chunks (PE) +
   ACT Identity bias copy (bf16); vm token-major via vT-stationary
   matmuls with a ones column appended (softmax denominator trick)
 - per 512-query chunk, software-pipelined over 128-key tiles:
   scores pair-packed into PE row groups 0-63/64-127 writing one
   2-bank PSUM tile, one [128,1024] ACT exp (scale=1/sqrt(hd), bf16),
   ctx_aug accumulated over key tiles; scores(kt) issue ahead of
   ctx(kt-1) so the PE never waits on ACT
 - 1/den = exp(-ln(den)) on ACT (same table set as the scores exp),
   K=1 matmul broadcast, DVE normalize multiply -> ctxn bf16
 - out-projection per 128 tokens into a shared 2-bank PSUM tile,
   DVE copy, DMA out (fp32 partials)
"""

import numpy as np
import ml_dtypes

import concourse.bass as bass
import concourse.mybir as mybir
import concourse.tile as tile
from concourse import bass_utils

N_CORES = 8
B, L, D = 2, 2048, 1024
NH, HD = 16, 64
HPC = NH // (N_CORES // B)  # 4 heads per core
CW = HPC * HD  # 256 features per core
TC = L  # tokens per core (one batch)
SCALE = 1.0 / np.sqrt(HD)
DC = D // 128  # 8 contraction tiles for the projections
NT = TC // 128  # 16 key tiles
NCH = TC // 512  # 4 query chunks

F32 = mybir.dt.float32
BF16 = mybir.dt.bfloat16


def _split_matmul_waits(nc):
    """Instructions whose ISA struct has a single sem-wait slot (matmul
    self-loading LDW, HWDGE DMA) reject >1 wait in walrus. Move extra
    waits onto same-engine NoOps inserted right before (program order on
    the sequencer preserves the happens-before)."""
    for f in nc.m.functions:
        for bb in f.blocks:
            insts = list(bb.instructions)
            out = []
            for inst in insts:
                si = inst.sync_info
                if si is not None and len(si.on_wait) > 1:
                    for w in si.on_wait[:-1]:
                        nop = mybir.InstNoOp(
                            name=nc.get_next_instruction_name(),
                            ins=[],
                            outs=[],
                            engine=inst.engine,
                            bass_nofuse=True,
                        )
                        nop.sync_info = mybir.SyncInfo(on_wait=[w], on_update=[])
                        out.append(nop)
                    inst.sync_info = mybir.SyncInfo(
                        on_wait=[si.on_wait[-1]], on_update=si.on_update
                    )
                out.append(inst)
            if len(out) != len(insts):
                bb.instructions = out
    return nc


def build_nc():
    nc = bass.Bass("TRN2", target_bir_lowering=False, debug=False)

    qtf = nc.dram_tensor("qtf", [D, TC], BF16, kind="ExternalInput").ap()
    ktf = nc.dram_tensor("ktf", [D, TC], BF16, kind="ExternalInput").ap()
    vtf = nc.dram_tensor("vtf", [D, TC], BF16, kind="ExternalInput").ap()
    wqt = nc.dram_tensor("wqt", [D, CW], BF16, kind="ExternalInput").ap()
    wkt = nc.dram_tensor("wkt", [D, CW], BF16, kind="ExternalInput").ap()
    wvt = nc.dram_tensor("wvt", [D, CW], BF16, kind="ExternalInput").ap()
    wot = nc.dram_tensor("wot", [CW, D], BF16, kind="ExternalInput").ap()
    bq = nc.dram_tensor("bq", [1, CW], BF16, kind="ExternalInput").ap()
    bk = nc.dram_tensor("bk", [1, CW], BF16, kind="ExternalInput").ap()
    bv = nc.dram_tensor("bv", [1, CW], BF16, kind="ExternalInput").ap()
    out_p = nc.dram_tensor("out_p", [TC, D], BF16, kind="ExternalOutput").ap()

    with tile.TileContext(nc) as tc:
        with (
            tc.tile_pool(name="singles", bufs=1) as singles,
            tc.tile_pool(name="acts", bufs=1) as acts,
            tc.tile_pool(name="small", bufs=4) as small,
            tc.tile_pool(name="es", bufs=2) as esp,
            tc.tile_pool(name="psum", bufs=1, space="PSUM") as pp,
        ):
            # ---- weights / biases / transposed activations -> SBUF ----
            # split the big loads into per-k-tile DMAs alternating the two
            # HWDGE queues so they spread across DMA engines
            nq = 0

            def ldma(dst, src):
                nonlocal nq
                eng = nc.sync if nq % 2 == 0 else nc.scalar
                nq += 1
                eng.dma_start(dst, src)

            # weights/biases first so the first projection can start as soon
            # as the k slices land
            w_sb = {}
            for name, dram in (("wq", wqt), ("wk", wkt), ("wv", wvt)):
                w = singles.tile([128, DC, CW], BF16, name=name + "_sb")
                ldma(w, dram.rearrange("(c p) h -> p c h", p=128))
                w_sb[name] = w
            wot_sb = singles.tile([128, CW // 128, D], BF16)
            ldma(wot_sb, wot.rearrange("(c p) d -> p c d", p=128))
            b_sb = {}
            for name, dram in (("bq", bq), ("bk", bk), ("bv", bv)):
                b = singles.tile([1, CW], BF16, name=name + "_sb")
                ldma(b, dram)
                b_sb[name] = b
            xT = {}
            for name, dram in (("k", ktf), ("v", vtf), ("q", qtf)):
                t = singles.tile([128, DC, TC], BF16, name=name + "T_sb")
                for dc in range(DC):
                    ldma(t[:, dc, :], dram[dc * 128 : (dc + 1) * 128, :])
                xT[name] = t
            onecol = singles.tile([1, 128], BF16)
            nc.vector.memset(onecol, 1.0)
            ones512 = singles.tile([1, 512], BF16)
            nc.vector.memset(ones512, 1.0)
            o65 = singles.tile([65, 64], BF16)
            nc.vector.memset(o65[64:65, :], 1.0)

            # ---- activations ----
            qm = acts.tile([128, CW // 128, TC], BF16)  # feature-major
            km = acts.tile([128, CW // 128, TC], BF16)
            vma = acts.tile([128, NT, HPC * 65], BF16)  # [t%128, t//128, (hv|1)x4]
            ctxn = acts.tile([128, CW // 128, TC], BF16)

            vma_g = vma.rearrange("p t (x c) -> p t x c", c=65)
            nc.vector.memset(vma_g[:, :, :, 64], 1.0)

            def proj_qk_half(src, wname, dst, ci, bias, c):
                # one feature-half of a 512-token chunk projection; bias is
                # folded in as a K=1 rank-1 matmul so the PSUM drain is a
                # plain DVE copy (keeps ACT free for the exp stream)
                ts = slice(ci * 512, (ci + 1) * 512)
                ps = pp.tile([128, 1024], F32, tag="s", bufs=2)
                psl = ps[:, 0:512]
                nc.tensor.matmul(
                    psl,
                    lhsT=bias[:, c * 128 : (c + 1) * 128],
                    rhs=ones512,
                    start=True,
                    stop=False,
                )
                for dc in range(DC):
                    nc.tensor.matmul(
                        psl,
                        lhsT=w_sb[wname][:, dc, c * 128 : (c + 1) * 128],
                        rhs=xT[src][:, dc, ts],
                        start=False,
                        stop=(dc == DC - 1),
                    )
                nc.vector.tensor_copy(dst[:, c, ts], psl)

            def proj_qk(src, wname, dst, ci, bias):
                for c in range(CW // 128):
                    proj_qk_half(src, wname, dst, ci, bias, c)

            def proj_v_half(g, half):
                # token-major vm for two key tiles of group g
                ps = pp.tile([128, 1024], F32, tag="s", bufs=2)
                for j in range(2):
                    tt = half * 2 + j
                    t0 = g * 512 + tt * 128
                    psl = ps[:, j * 256 : (j + 1) * 256]
                    nc.tensor.matmul(
                        psl, lhsT=onecol, rhs=b_sb["bv"], start=True, stop=False
                    )
                    for dc in range(DC):
                        nc.tensor.matmul(
                            psl,
                            lhsT=xT["v"][:, dc, t0 : t0 + 128],
                            rhs=w_sb["wv"][:, dc, :],
                            start=False,
                            stop=(dc == DC - 1),
                        )
                vg = vma_g[:, g * 4 + half * 2 : g * 4 + half * 2 + 2]
                nc.vector.tensor_copy(
                    vg[:, :, :, 0:64],
                    ps[:, 0:512].rearrange("p (t x c) -> p t x c", t=2, c=64),
                )

            def proj_kv_group(g):
                proj_qk("k", "wk", km, g, b_sb["bk"])
                proj_v_half(g, 0)
                proj_v_half(g, 1)

            # ---- normalize + out-projection for a finished chunk ----
            def finish_chunk(lc, ctx):
                qs = slice(lc * 512, (lc + 1) * 512)
                # softmax denominators: 1/den = exp(-ln(den)) on ACT
                for h in range(HPC):
                    lnd = small.tile([65, 512], F32, tag="lnd")
                    nc.scalar.activation(
                        lnd[64:65, :],
                        ctx[h][64:65, :],
                        mybir.ActivationFunctionType.Ln,
                    )
                    rc4 = small.tile([65, 512], BF16, tag="rc4")
                    with nc.allow_low_precision(reason="bf16 softmax denom"):
                        nc.scalar.activation(
                            rc4[64:65, :],
                            lnd[64:65, :],
                            mybir.ActivationFunctionType.Exp,
                            scale=-1.0,
                        )
                    nc.tensor.matmul(
                        ctx[h][64:128, :], lhsT=o65[64:65, :], rhs=rc4[64:65, :]
                    )
                    bcs = small.tile([64, 512], F32, tag="bcs")
                    nc.vector.tensor_copy(bcs, ctx[h][64:128, :])
                    nc.vector.tensor_mul(
                        ctxn[(h % 2) * 64 : (h % 2) * 64 + 64, h // 2, qs],
                        ctx[h][0:64, :],
                        bcs,
                    )
                for tt in range(4):
                    t0 = lc * 512 + tt * 128
                    ob = small.tile([128, D], BF16, tag="ob")
                    po = pp.tile([128, 1024], F32, tag="s", bufs=2)
                    for eh in range(2):
                        for c in range(CW // 128):
                            nc.tensor.matmul(
                                po[:, eh * 512 : (eh + 1) * 512],
                                lhsT=ctxn[:, c, t0 : t0 + 128],
                                rhs=wot_sb[:, c, eh * 512 : (eh + 1) * 512],
                                start=(c == 0),
                                stop=(c == CW // 128 - 1),
                            )
                    nc.vector.tensor_copy(ob, po)
                    nc.sync.dma_start(out_p[t0 : t0 + 128, :], ob)

            # ---- attention for one query chunk ----
            # The previous chunk's normalize/out-projection is emitted after
            # this chunk's first two score tiles, so its ACT recip chain and
            # PE broadcast/out-proj matmuls overlap fresh exp/score work
            # instead of stalling the pipe at the chunk boundary. ctx tiles
            # are allocated after the previous chunk's are released.
            def attention_chunk(lc, finish_prev=None, interleave=None):
                qs = slice(lc * 512, (lc + 1) * 512)

                def emit_scores(kt):
                    ks = slice(kt * 128, (kt + 1) * 128)
                    es = []
                    for pr in range(HPC // 2):
                        s = pp.tile([128, 1024], F32, tag="s", bufs=2)
                        for hh in range(2):
                            hp = slice(hh * 64, hh * 64 + 64)
                            nc.tensor.matmul(
                                s[:, hh * 512 : (hh + 1) * 512],
                                lhsT=km[hp, pr, ks],
                                rhs=qm[hp, pr, qs],
                                tile_position=(hh * 64, 0),
                            )
                        e = esp.tile([128, 1024], BF16, tag=f"e{pr}", bufs=3)
                        nc.scalar.activation(
                            e, s, mybir.ActivationFunctionType.Exp, scale=SCALE
                        )
                        es.append(e)
                    return es

                ctx = []

                def emit_ctx(kt, es):
                    for h in range(HPC):
                        nc.tensor.matmul(
                            ctx[h][0:65, :],
                            lhsT=vma[:, kt, h * 65 : h * 65 + 65],
                            rhs=es[h // 2][:, (h % 2) * 512 : (h % 2) * 512 + 512],
                            start=(kt == 0),
                            stop=(kt == NT - 1),
                        )

                e0 = emit_scores(0)
                e1 = emit_scores(1)
                if finish_prev is not None:
                    finish_prev()
                ctx.extend(
                    pp.tile([128, 512], F32, tag="ctx", bufs=4, name=f"ctx{h}")
                    for h in range(HPC)
                )
                emit_ctx(0, e0)
                prev = e1
                for kt in range(2, NT):
                    if interleave is not None and kt in interleave:
                        interleave[kt]()
                    cur = emit_scores(kt)
                    emit_ctx(kt - 1, prev)
                    prev = cur
                emit_ctx(NT - 1, prev)
                return ctx

            # ---- schedule: kv group 0 + q chunk 0, then stream the rest.
            # chunk 0's interleave spreads each kv group over four kt slots
            # (~2-4us PE bursts) so the exp stream never runs dry; group g+1
            # finishes just before its first key tile (kt=4(g+1)) is needed.
            def kmh(g, c):
                return lambda: proj_qk_half("k", "wk", km, g, b_sb["bk"], c)

            def vmh(g, half):
                return lambda: proj_v_half(g, half)

            def qmh(ci, c):
                return lambda: proj_qk_half("q", "wq", qm, ci, b_sb["bq"], c)

            proj_kv_group(0)
            proj_qk("q", "wq", qm, 0, b_sb["bq"])
            hooks0 = {}
            for g in (1, 2, 3):
                base = 4 * (g - 1) + 2
                hooks0[base] = kmh(g, 0)
                hooks0[base + 1] = kmh(g, 1)
                hooks0[base + 2] = vmh(g, 0)
                hooks0[base + 3] = vmh(g, 1)
            hooks0[14] = qmh(1, 0)
            hooks0[15] = qmh(1, 1)
            ctx_prev = attention_chunk(0, interleave=hooks0)
            for ci in range(1, NCH):
                hooks = (
                    {13: qmh(ci + 1, 0), 14: qmh(ci + 1, 1)} if ci + 1 < NCH else None
                )
                ctx_prev = attention_chunk(
                    ci,
                    finish_prev=(lambda lc=ci - 1, c=ctx_prev: finish_chunk(lc, c)),
                    interleave=hooks,
                )
            finish_chunk(NCH - 1, ctx_prev)
    return _split_matmul_waits(nc)


_NC_CACHE = None
LAST_RESULTS = None


def kernel(q, k, v, attention_mask, Wq, bq, Wk, bk, Wv, bv, Wo, bo):
    global _NC_CACHE, LAST_RESULTS
    assert np.asarray(attention_mask).all(), "kernel assumes all-ones mask"
    if _NC_CACHE is None:
        _NC_CACHE = build_nc()
    nc = _NC_CACHE

    bfc = lambda x: np.ascontiguousarray(np.asarray(x, np.float32)).astype(
        ml_dtypes.bfloat16
    )
    c = np.ascontiguousarray
    qTb = [bfc(np.asarray(q, np.float32)[b].T) for b in range(B)]
    kTb = [bfc(np.asarray(k, np.float32)[b].T) for b in range(B)]
    vTb = [bfc(np.asarray(v, np.float32)[b].T) for b in range(B)]
    WqT = np.asarray(Wq, np.float32).T
    WkT = np.asarray(Wk, np.float32).T
    WvT = np.asarray(Wv, np.float32).T
    WoT = np.asarray(Wo, np.float32).T

    in_maps = []
    for ci in range(N_CORES):
        b = ci // (N_CORES // B)
        g = ci % (N_CORES // B)
        hs = slice(g * CW, (g + 1) * CW)
        in_maps.append(
            {
                "qtf": qTb[b],
                "ktf": kTb[b],
                "vtf": vTb[b],
                "wqt": bfc(WqT[:, hs]),
                "wkt": bfc(WkT[:, hs]),
                "wvt": bfc(WvT[:, hs]),
                "wot": bfc(WoT[hs, :]),
                "bq": bfc(np.asarray(bq, np.float32)[None, hs]),
                "bk": bfc(np.asarray(bk, np.float32)[None, hs]),
                "bv": bfc(np.asarray(bv, np.float32)[None, hs]),
            }
        )

    res = bass_utils.run_bass_kernel_spmd(nc, in_maps, core_ids=list(range(N_CORES)))
    LAST_RESULTS = res
    out = np.zeros((B, L, D), np.float32)
    for ci, r in enumerate(res.results):
        out[ci // (N_CORES // B)] += np.asarray(r["out_p"], np.float32)
    out += np.asarray(bo, np.float32)[None, None, :]
    return out


# revision 4
# speedup vs baseline: 1.0039x; 1.0039x over previous
"""Multi-head cross-attention (MHAForCrossFusion) on 8 Trainium2 cores.

Final: hybrid batch x head tensor-parallel (cores 0-3 batch 0, cores 4-7
batch 1; 4 heads / 256 features per core), all matmuls bf16, host sums
the 4 partial out-projections per batch (+ bo).

q/k/v are transposed to feature-major [D, L] on the HOST (like the
weight layouts), so the device sees plain fast DMAs instead of XBAR
DMA-transposes -- the q/k/v feature-major tiles live resident in SBUF.

Device program per core:
 - qm/km feature-major [256f, 2048t] = Wx_slice.T @ xT chunks (PE) +
   ACT Identity bias copy (bf16); vm token-major via vT-stationary
   matmuls with a ones column appended (softmax denominator trick)
 - per 512-query chunk, software-pipelined over 128-key tiles:
   scores pair-packed into PE row groups 0-63/64-127 writing one
   2-bank PSUM tile, one [128,1024] ACT exp (scale=1/sqrt(hd), bf16),
   ctx_aug accumulated over key tiles; scores(kt) issue ahead of
   ctx(kt-1) so the PE never waits on ACT
 - 1/den = exp(-ln(den)) on ACT (same table set as the scores exp),
   K=1 matmul broadcast, DVE normalize multiply -> ctxn bf16
 - out-projection per 128 tokens into a shared 2-bank PSUM tile,
   DVE copy, DMA out (fp32 partials)
"""

import numpy as np
import ml_dtypes

import concourse.bass as bass
import concourse.mybir as mybir
import concourse.tile as tile
from concourse import bass_utils

N_CORES = 8
B, L, D = 2, 2048, 1024
NH, HD = 16, 64
HPC = NH // (N_CORES // B)  # 4 heads per core
CW = HPC * HD  # 256 features per core
TC = L  # tokens per core (one batch)
SCALE = 1.0 / np.sqrt(HD)
DC = D // 128  # 8 contraction tiles for the projections
NT = TC // 128  # 16 key tiles
NCH = TC // 512  # 4 query chunks

F32 = mybir.dt.float32
BF16 = mybir.dt.bfloat16


def _split_matmul_waits(nc):
    """Instructions whose ISA struct has a single sem-wait slot (matmul
    self-loading LDW, HWDGE DMA) reject >1 wait in walrus. Move extra
    waits onto same-engine NoOps inserted right before (program order on
    the sequencer preserves the happens-before)."""
    for f in nc.m.functions:
        for bb in f.blocks:
            insts = list(bb.instructions)
            out = []
            for inst in insts:
                si = inst.sync_info
                if si is not None and len(si.on_wait) > 1:
                    for w in si.on_wait[:-1]:
                        nop = mybir.InstNoOp(
                            name=nc.get_next_instruction_name(),
                            ins=[],
                            outs=[],
                            engine=inst.engine,
                            bass_nofuse=True,
                        )
                        nop.sync_info = mybir.SyncInfo(on_wait=[w], on_update=[])
                        out.append(nop)
                    inst.sync_info = mybir.SyncInfo(
                        on_wait=[si.on_wait[-1]], on_update=si.on_update
                    )
                out.append(inst)
            if len(out) != len(insts):
                bb.instructions = out
    return nc


def build_nc():
    nc = bass.Bass("TRN2", target_bir_lowering=False, debug=False)

    qtf = nc.dram_tensor("qtf", [D, TC], BF16, kind="ExternalInput").ap()
    ktf = nc.dram_tensor("ktf", [D, TC], BF16, kind="ExternalInput").ap()
    vtf = nc.dram_tensor("vtf", [D, TC], BF16, kind="ExternalInput").ap()
    wqt = nc.dram_tensor("wqt", [D, CW], BF16, kind="ExternalInput").ap()
    wkt = nc.dram_tensor("wkt", [D, CW], BF16, kind="ExternalInput").ap()
    wvt = nc.dram_tensor("wvt", [D, CW], BF16, kind="ExternalInput").ap()
    wot = nc.dram_tensor("wot", [CW, D], BF16, kind="ExternalInput").ap()
    bq = nc.dram_tensor("bq", [1, CW], BF16, kind="ExternalInput").ap()
    bk = nc.dram_tensor("bk", [1, CW], BF16, kind="ExternalInput").ap()
    bv = nc.dram_tensor("bv", [1, CW], BF16, kind="ExternalInput").ap()
    out_p = nc.dram_tensor("out_p", [TC, D], BF16, kind="ExternalOutput").ap()

    with tile.TileContext(nc) as tc:
        with (
            tc.tile_pool(name="singles", bufs=1) as singles,
            tc.tile_pool(name="acts", bufs=1) as acts,
            tc.tile_pool(name="small", bufs=4) as small,
            tc.tile_pool(name="es", bufs=2) as esp,
            tc.tile_pool(name="psum", bufs=1, space="PSUM") as pp,
        ):
            # ---- weights / biases / transposed activations -> SBUF ----
            # split the big loads into per-k-tile DMAs alternating the two
            # HWDGE queues so they spread across DMA engines
            nq = 0

            def ldma(dst, src):
                nonlocal nq
                eng = nc.sync if nq % 2 == 0 else nc.scalar
                nq += 1
                eng.dma_start(dst, src)

            # load order: the k-projection inputs (wk, bk, k slices) first so
            # the first km matmul can start ~10us in; everything else follows
            w_sb = {
                name: singles.tile([128, DC, CW], BF16, name=name + "_sb")
                for name in ("wq", "wk", "wv")
            }
            wot_sb = singles.tile([128, CW // 128, D], BF16)
            b_sb = {
                name: singles.tile([1, CW], BF16, name=name + "_sb")
                for name in ("bq", "bk", "bv")
            }
            xT = {
                name: singles.tile([128, DC, TC], BF16, name=name + "T_sb")
                for name in ("k", "v", "q")
            }
            # k then q first (the prologue projections need both before the
            # first scores); v trails (only ctx(0) needs it)
            ldma(w_sb["wk"], wkt.rearrange("(c p) h -> p c h", p=128))
            ldma(b_sb["bk"], bk)
            ldma(w_sb["wq"], wqt.rearrange("(c p) h -> p c h", p=128))
            ldma(b_sb["bq"], bq)
            for dc in range(DC):
                ldma(xT["k"][:, dc, :], ktf[dc * 128 : (dc + 1) * 128, :])
                ldma(xT["q"][:, dc, :], qtf[dc * 128 : (dc + 1) * 128, :])
            ldma(w_sb["wv"], wvt.rearrange("(c p) h -> p c h", p=128))
            ldma(b_sb["bv"], bv)
            for dc in range(DC):
                ldma(xT["v"][:, dc, :], vtf[dc * 128 : (dc + 1) * 128, :])
            ldma(wot_sb, wot.rearrange("(c p) d -> p c d", p=128))
            onecol = singles.tile([1, 128], BF16)
            nc.vector.memset(onecol, 1.0)
            ones512 = singles.tile([1, 512], BF16)
            nc.vector.memset(ones512, 1.0)
            o65 = singles.tile([65, 64], BF16)
            nc.vector.memset(o65[64:65, :], 1.0)

            # ---- activations ----
            qm = acts.tile([128, CW // 128, TC], BF16)  # feature-major
            km = acts.tile([128, CW // 128, TC], BF16)
            vma = acts.tile([128, NT, HPC * 65], BF16)  # [t%128, t//128, (hv|1)x4]
            ctxn = acts.tile([128, CW // 128, TC], BF16)

            vma_g = vma.rearrange("p t (x c) -> p t x c", c=65)
            nc.vector.memset(vma_g[:, :, :, 64], 1.0)

            def proj_qk_half(src, wname, dst, ci, bias, c):
                # one feature-half of a 512-token chunk projection; bias is
                # folded in as a K=1 rank-1 matmul so the PSUM drain is a
                # plain DVE copy (keeps ACT free for the exp stream)
                ts = slice(ci * 512, (ci + 1) * 512)
                ps = pp.tile([128, 1024], F32, tag="s", bufs=2)
                psl = ps[:, 0:512]
                nc.tensor.matmul(
                    psl,
                    lhsT=bias[:, c * 128 : (c + 1) * 128],
                    rhs=ones512,
                    start=True,
                    stop=False,
                )
                for dc in range(DC):
                    nc.tensor.matmul(
                        psl,
                        lhsT=w_sb[wname][:, dc, c * 128 : (c + 1) * 128],
                        rhs=xT[src][:, dc, ts],
                        start=False,
                        stop=(dc == DC - 1),
                    )
                nc.vector.tensor_copy(dst[:, c, ts], psl)

            def proj_qk(src, wname, dst, ci, bias):
                for c in range(CW // 128):
                    proj_qk_half(src, wname, dst, ci, bias, c)

            def proj_v_half(g, half):
                # token-major vm for two key tiles of group g
                ps = pp.tile([128, 1024], F32, tag="s", bufs=2)
                for j in range(2):
                    tt = half * 2 + j
                    t0 = g * 512 + tt * 128
                    psl = ps[:, j * 256 : (j + 1) * 256]
                    nc.tensor.matmul(
                        psl, lhsT=onecol, rhs=b_sb["bv"], start=True, stop=False
                    )
                    for dc in range(DC):
                        nc.tensor.matmul(
                            psl,
                            lhsT=xT["v"][:, dc, t0 : t0 + 128],
                            rhs=w_sb["wv"][:, dc, :],
                            start=False,
                            stop=(dc == DC - 1),
                        )
                vg = vma_g[:, g * 4 + half * 2 : g * 4 + half * 2 + 2]
                nc.vector.tensor_copy(
                    vg[:, :, :, 0:64],
                    ps[:, 0:512].rearrange("p (t x c) -> p t x c", t=2, c=64),
                )

            def proj_kv_group(g):
                proj_qk("k", "wk", km, g, b_sb["bk"])
                proj_v_half(g, 0)
                proj_v_half(g, 1)

            # ---- normalize + out-projection for a finished chunk ----
            def finish_chunk(lc, ctx):
                qs = slice(lc * 512, (lc + 1) * 512)
                # softmax denominators: 1/den = exp(-ln(den)) on ACT -- short
                # per-head latency, and it fills ACT's boundary bubble
                for h in range(HPC):
                    lnd = small.tile([65, 512], F32, tag="lnd")
                    nc.scalar.activation(
                        lnd[64:65, :],
                        ctx[h][64:65, :],
                        mybir.ActivationFunctionType.Ln,
                    )
                    rc4 = small.tile([65, 512], BF16, tag="rc4")
                    with nc.allow_low_precision(reason="bf16 softmax denom"):
                        nc.scalar.activation(
                            rc4[64:65, :],
                            lnd[64:65, :],
                            mybir.ActivationFunctionType.Exp,
                            scale=-1.0,
                        )
                    nc.tensor.matmul(
                        ctx[h][64:128, :], lhsT=o65[64:65, :], rhs=rc4[64:65, :]
                    )
                    bcs = small.tile([64, 512], F32, tag="bcs")
                    nc.vector.tensor_copy(bcs, ctx[h][64:128, :])
                    nc.vector.tensor_mul(
                        ctxn[(h % 2) * 64 : (h % 2) * 64 + 64, h // 2, qs],
                        ctx[h][0:64, :],
                        bcs,
                    )
                for tt in range(4):
                    t0 = lc * 512 + tt * 128
                    ob = small.tile([128, D], BF16, tag="ob")
                    po = pp.tile([128, 1024], F32, tag="s", bufs=2)
                    for eh in range(2):
                        for c in range(CW // 128):
                            nc.tensor.matmul(
                                po[:, eh * 512 : (eh + 1) * 512],
                                lhsT=ctxn[:, c, t0 : t0 + 128],
                                rhs=wot_sb[:, c, eh * 512 : (eh + 1) * 512],
                                start=(c == 0),
                                stop=(c == CW // 128 - 1),
                            )
                    nc.vector.tensor_copy(ob, po)
                    nc.sync.dma_start(out_p[t0 : t0 + 128, :], ob)

            # ---- attention for one query chunk ----
            # The previous chunk's normalize/out-projection is emitted after
            # this chunk's first two score tiles, so its ACT recip chain and
            # PE broadcast/out-proj matmuls overlap fresh exp/score work
            # instead of stalling the pipe at the chunk boundary. ctx tiles
            # are allocated after the previous chunk's are released.
            def attention_chunk(lc, pre_ctx=(), interleave=None):
                # pre_ctx closures are emitted after the first two score
                # tiles and before ctx allocation: independent PE work
                # (vm/qm projections) plus the previous chunk's finish --
                # they cover the softmax-recip chain latency so the ctx
                # ring handoff never stalls the pipe
                qs = slice(lc * 512, (lc + 1) * 512)

                def emit_scores(kt):
                    ks = slice(kt * 128, (kt + 1) * 128)
                    es = []
                    for pr in range(HPC // 2):
                        s = pp.tile([128, 1024], F32, tag="s", bufs=2)
                        for hh in range(2):
                            hp = slice(hh * 64, hh * 64 + 64)
                            nc.tensor.matmul(
                                s[:, hh * 512 : (hh + 1) * 512],
                                lhsT=km[hp, pr, ks],
                                rhs=qm[hp, pr, qs],
                                tile_position=(hh * 64, 0),
                            )
                        e = esp.tile([128, 1024], BF16, tag=f"e{pr}", bufs=3)
                        nc.scalar.activation(
                            e, s, mybir.ActivationFunctionType.Exp, scale=SCALE
                        )
                        es.append(e)
                    return es

                ctx = []

                def emit_ctx(kt, es):
                    for h in range(HPC):
                        nc.tensor.matmul(
                            ctx[h][0:65, :],
                            lhsT=vma[:, kt, h * 65 : h * 65 + 65],
                            rhs=es[h // 2][:, (h % 2) * 512 : (h % 2) * 512 + 512],
                            start=(kt == 0),
                            stop=(kt == NT - 1),
                        )

                e0 = emit_scores(0)
                e1 = emit_scores(1)
                for fn in pre_ctx:
                    fn()
                ctx.extend(
                    pp.tile([128, 512], F32, tag="ctx", bufs=4, name=f"ctx{h}")
                    for h in range(HPC)
                )
                emit_ctx(0, e0)
                prev = e1
                for kt in range(2, NT):
                    if interleave is not None and kt in interleave:
                        interleave[kt]()
                    cur = emit_scores(kt)
                    emit_ctx(kt - 1, prev)
                    prev = cur
                emit_ctx(NT - 1, prev)
                return ctx

            # ---- schedule: kv group 0 + q chunk 0, then stream the rest.
            # chunk 0's interleave spreads each kv group over four kt slots
            # (~2-4us PE bursts) so the exp stream never runs dry; group g+1
            # finishes just before its first key tile (kt=4(g+1)) is needed.
            def kmh(g, c):
                return lambda: proj_qk_half("k", "wk", km, g, b_sb["bk"], c)

            def vmh(g, half):
                return lambda: proj_v_half(g, half)

            def qmh(ci, c):
                return lambda: proj_qk_half("q", "wq", qm, ci, b_sb["bq"], c)

            # minimal prologue: only km group 0 and qm chunk 0 gate the
            # first scores; vm group 0 slots in right after them
            proj_qk("k", "wk", km, 0, b_sb["bk"])
            proj_qk("q", "wq", qm, 0, b_sb["bq"])
            hooks0 = {}
            for g in (1, 2, 3):
                base = 4 * (g - 1) + 2
                hooks0[base] = kmh(g, 0)
                hooks0[base + 1] = kmh(g, 1)
                hooks0[base + 2] = vmh(g, 0)
                hooks0[base + 3] = vmh(g, 1)
            hooks0[14] = qmh(1, 0)
            hooks0[15] = qmh(1, 1)
            ctx_prev = attention_chunk(
                0, pre_ctx=(vmh(0, 0), vmh(0, 1)), interleave=hooks0
            )
            for ci in range(1, NCH):
                pre = []
                if ci + 1 < NCH:
                    pre += [qmh(ci + 1, 0), qmh(ci + 1, 1)]
                pre.append(lambda lc=ci - 1, c=ctx_prev: finish_chunk(lc, c))
                ctx_prev = attention_chunk(ci, pre_ctx=pre)
            finish_chunk(NCH - 1, ctx_prev)
    return _split_matmul_waits(nc)


_NC_CACHE = None
LAST_RESULTS = None


def kernel(q, k, v, attention_mask, Wq, bq, Wk, bk, Wv, bv, Wo, bo):
    global _NC_CACHE, LAST_RESULTS
    assert np.asarray(attention_mask).all(), "kernel assumes all-ones mask"
    if _NC_CACHE is None:
        _NC_CACHE = build_nc()
    nc = _NC_CACHE

    bfc = lambda x: np.ascontiguousarray(np.asarray(x, np.float32)).astype(
        ml_dtypes.bfloat16
    )
    c = np.ascontiguousarray
    qTb = [bfc(np.asarray(q, np.float32)[b].T) for b in range(B)]
    kTb = [bfc(np.asarray(k, np.float32)[b].T) for b in range(B)]
    vTb = [bfc(np.asarray(v, np.float32)[b].T) for b in range(B)]
    WqT = np.asarray(Wq, np.float32).T
    WkT = np.asarray(Wk, np.float32).T
    WvT = np.asarray(Wv, np.float32).T
    WoT = np.asarray(Wo, np.float32).T

    in_maps = []
    for ci in range(N_CORES):
        b = ci // (N_CORES // B)
        g = ci % (N_CORES // B)
        hs = slice(g * CW, (g + 1) * CW)
        in_maps.append(
            {
                "qtf": qTb[b],
                "ktf": kTb[b],
                "vtf": vTb[b],
                "wqt": bfc(WqT[:, hs]),
                "wkt": bfc(WkT[:, hs]),
                "wvt": bfc(WvT[:, hs]),
                "wot": bfc(WoT[hs, :]),
                "bq": bfc(np.asarray(bq, np.float32)[None, hs]),
                "bk": bfc(np.asarray(bk, np.float32)[None, hs]),
                "bv": bfc(np.asarray(bv, np.float32)[None, hs]),
            }
        )

    res = bass_utils.run_bass_kernel_spmd(nc, in_maps, core_ids=list(range(N_CORES)))
    LAST_RESULTS = res
    out = np.zeros((B, L, D), np.float32)
    for ci, r in enumerate(res.results):
        out[ci // (N_CORES // B)] += np.asarray(r["out_p"], np.float32)
    out += np.asarray(bo, np.float32)[None, None, :]
    return out
